# revision 8
# baseline (speedup 1.0000x reference)
"""LoRA linear layer on 8 Trainium2 NeuronCores.

Computes out = x @ (lora_B @ lora_A * 2).T + bias for
x [4, 2048, 4096], lora_A [16, 4096], lora_B [4096, 16], bias [4096].

Strategy: pure data parallel — shard x over batch*seq (8192 rows -> 1024
rows/core), replicate the tiny LoRA weights. Exploit the rank-16 structure:
y = x @ A^T (contract 4096), z = y @ B^T + bias (contract 16+1 via the
ones-row trick), never materializing the dense 4096x4096 W.

Everything on the wire is fp16 (gate is rel_err < 2e-2; measured fp16
end-to-end error ~6e-4): x is cast AND pre-transposed on the host into the
exact SBUF layout [128 partitions = feature%128, (group, k-chunk, row)],
so the device does zero transposes and both GEMMs run at full 16-bit PE
rate with fp32 PSUM accumulation. The output travels back as fp16 and is
upcast on the host. Per-core HBM traffic: 8.4 MiB in + 8.4 MiB out.

Scheduling notes (from trace analysis):
  - Concurrent DMAs queued on one HWDGE ring interleave at packet level
    and complete near-simultaneously, so inputs are split into 1 MiB
    pieces whose ENQUEUE is paced: a 16-byte SBUF->SBUF "token" DMA
    reads piece i-2 and writes into piece i's tile ahead of the real
    load, so piece i's dispatch waits (WAW) for piece i-2's completion.
    Pacing is thus tied to DMA completion order only — not to PE
    progress — and pieces land every ~2.5 us at full ring rate.
  - Outputs go out on the second HWDGE ring (nc.scalar) so they do not
    queue behind inputs on the SP ring. SWDGE (gpsimd) measured only
    ~205 GB/s — not used.
  - PSUM->SBUF copies are [128, 1024] (2 PSUM banks) per instruction,
    alternating DVE/ACT, to amortize per-instruction overhead.
  - ~64 tiny garbage matmuls run during the initial DMA wait so the PE
    HAM clock-gate is already released (2.4 GHz) when GEMM1 starts.
"""

import sys

import numpy as np

if "/opt/trn_rl_repo" not in sys.path:
    sys.path.insert(0, "/opt/trn_rl_repo")

import concourse.bass as bass
import concourse.mybir as mybir
from concourse import bacc
from concourse.bass_utils import run_bass_kernel_spmd
from concourse.tile import TileContext

N_CORES = 8
B, S, IN_F, OUT_F, R = 4, 2048, 4096, 4096, 16
ROWS = B * S // N_CORES  # 1024 rows per core
SCALING = 2.0  # alpha / r = 32 / 16, folded into A on the host
FP16 = mybir.dt.float16
FP32 = mybir.dt.float32
P = 128
NK = IN_F // P  # 32 contraction chunks for GEMM1
GROWS = 512  # rows per group (GEMM1 moving free dim)
NG = ROWS // GROWS  # 2 groups per core
HT = GROWS // P  # 4 row-tiles per group
ZC = 512  # GEMM2 moving chunk (PSUM bank = 512 fp32)
NJ = OUT_F // ZC  # 8 output chunks per row tile
PIECE_K = 8  # k-chunks per input DMA piece (1 MiB pieces)
NP = NK // PIECE_K  # 4 pieces per group
NPIECES = NG * NP  # 8 input pieces per core
NWARM = 64  # HAM warm-up matmuls

_nc_cache = None


def build_nc() -> bass.Bass:
    nc = bacc.Bacc()
    x_d = nc.declare_dram_parameter("xt", [P, NG * NK * GROWS], FP16, isOutput=False)
    a_d = nc.declare_dram_parameter("at", [P, NK * R], FP16, isOutput=False)
    bb_d = nc.declare_dram_parameter("bb", [R + 1, OUT_F], FP16, isOutput=False)
    out_d = nc.declare_dram_parameter("out", [ROWS, OUT_F], FP16, isOutput=True)

    with TileContext(nc) as tc:
        with (
            tc.tile_pool(name="const", bufs=1) as const,
            tc.tile_pool(name="xin", bufs=NPIECES) as xin,
            tc.tile_pool(name="zrp", bufs=3) as zrp,
            tc.tile_pool(name="ytp", bufs=2) as ytp,
            tc.tile_pool(name="ypsum", bufs=2, space="PSUM") as ypsum,
            tc.tile_pool(name="zpsum", bufs=3, space="PSUM") as zpsum,
        ):
            # HAM warm-up: keep the PE busy on garbage during the initial
            # DMA wait so real matmuls run at 2.4 GHz, not 1.2.
            warm_sb = const.tile([P, 64], FP16)
            nc.vector.memset(warm_sb[:, :], 0.0)
            warm_ps = zpsum.tile([P, 2 * ZC], FP32, tag="zz")
            for w in range(NWARM):
                nc.tensor.matmul(
                    warm_ps[0:64, 0:64],
                    lhsT=warm_sb[:, 0:64],
                    rhs=warm_sb[:, 0:64],
                    start=(w == 0),
                    stop=(w == NWARM - 1),
                )

            at_sb = const.tile([P, NK * R], FP16)
            nc.sync.dma_start(out=at_sb[:, :], in_=a_d[:, :])
            bb_sb = const.tile([R + 1, OUT_F], FP16)
            nc.sync.dma_start(out=bb_sb[:, :], in_=bb_d[:, :])

            PC = PIECE_K * GROWS  # columns per piece
            # Pre-allocate all input piece tiles, then emit loads in order
            # with token DMAs pacing the ring at depth 2.
            pieces = [
                xin.tile([P, PC], FP16, tag="x", name=f"xpiece{i}")
                for i in range(NPIECES)
            ]
            for i in range(NPIECES):
                if i >= 2:
                    # 16-byte token: read piece i-2, write into piece i's
                    # tile. Forces piece i's load to dispatch only after
                    # piece i-2 has fully landed.
                    nc.sync.dma_start(
                        out=pieces[i][0:1, 0:8], in_=pieces[i - 2][0:1, 0:8]
                    )
                nc.sync.dma_start(
                    out=pieces[i][:, :], in_=x_d[:, i * PC : (i + 1) * PC]
                )

            for g in range(NG):
                y_ps = ypsum.tile([R, GROWS], FP32, tag="y")
                for k in range(NK):
                    q, kk = k // PIECE_K, k % PIECE_K
                    nc.tensor.matmul(
                        y_ps,
                        lhsT=at_sb[:, k * R : (k + 1) * R],
                        rhs=pieces[g * NP + q][:, kk * GROWS : (kk + 1) * GROWS],
                        start=(k == 0),
                        stop=(k == NK - 1),
                    )

                # Ones-fill the whole tile (engines can't start at partition
                # 16), then overwrite rows 0:16 with y — row 16 keeps the 1.0.
                yt = ytp.tile([R + 1, GROWS], FP16, tag="yt")
                nc.vector.memset(yt[:, :], 1.0)
                nc.scalar.copy(out=yt[0:R, :], in_=y_ps)

                for h in range(HT):
                    zrow = zrp.tile([P, OUT_F], FP16, tag="z")
                    for jp in range(NJ // 2):
                        z_ps = zpsum.tile([P, 2 * ZC], FP32, tag="zz")
                        for half in range(2):
                            j = 2 * jp + half
                            nc.tensor.matmul(
                                z_ps[:, half * ZC : (half + 1) * ZC],
                                lhsT=yt[:, h * P : (h + 1) * P],
                                rhs=bb_sb[:, j * ZC : (j + 1) * ZC],
                                start=True,
                                stop=True,
                            )
                        dst = zrow[:, jp * 2 * ZC : (jp + 1) * 2 * ZC]
                        if jp % 2 == 0:
                            nc.vector.tensor_copy(out=dst, in_=z_ps[:, :])
                        else:
                            nc.scalar.copy(out=dst, in_=z_ps[:, :])
                    # Second HWDGE ring (ACT) — outputs never queue behind
                    # inputs on the SP ring.
                    nc.scalar.dma_start(
                        out=out_d[(g * HT + h) * P : (g * HT + h + 1) * P, :],
                        in_=zrow[:, :],
                    )

    nc.finalize()
    return nc


def make_in_maps(x, lora_A, lora_B, bias):
    f16 = np.float16
    x2 = np.asarray(x, dtype=np.float32).reshape(B * S, IN_F).astype(f16)
    a2 = (SCALING * np.asarray(lora_A, dtype=np.float32)).astype(f16)
    # at[p, k*16+r] = 2*A[r, k*128+p]
    at = np.ascontiguousarray(
        a2.reshape(R, NK, P).transpose(2, 1, 0).reshape(P, NK * R)
    )
    bb = np.ascontiguousarray(
        np.concatenate(
            [
                np.asarray(lora_B, dtype=np.float32).T.astype(f16),
                np.asarray(bias, dtype=np.float32).astype(f16).reshape(1, OUT_F),
            ],
            axis=0,
        )
    )
    in_maps = []
    for shard in np.split(x2, N_CORES, axis=0):  # [1024, 4096] each
        # xt[p, g, k, r] = shard[g*512 + r, k*128 + p]
        xt = np.ascontiguousarray(
            shard.reshape(NG, GROWS, NK, P)
            .transpose(3, 0, 2, 1)
            .reshape(P, NG * NK * GROWS)
        )
        in_maps.append({"xt": xt, "at": at, "bb": bb})
    return in_maps


def run(inputs: dict, trace: bool = False, **kw):
    global _nc_cache
    if _nc_cache is None:
        _nc_cache = build_nc()
    in_maps = make_in_maps(**inputs)
    res = run_bass_kernel_spmd(
        _nc_cache, in_maps, list(range(N_CORES)), trace=trace, **kw
    )
    out = (
        np.concatenate([res.results[i]["out"] for i in range(N_CORES)], axis=0)
        .astype(np.float32)
        .reshape(B, S, OUT_F)
    )
    return out, res


def kernel(**inputs) -> np.ndarray:
    out, _ = run(inputs)
    return out


# revision 9
# speedup vs baseline: 1.2940x; 1.2940x over previous
"""LoRA linear layer on 8 Trainium2 NeuronCores.

Computes out = x @ (lora_B @ lora_A * 2).T + bias for
x [4, 2048, 4096], lora_A [16, 4096], lora_B [4096, 16], bias [4096].

Strategy: pure data parallel — shard x over batch*seq (8192 rows -> 1024
rows/core), replicate the tiny LoRA weights. Exploit the rank-16 structure:
y = x @ A^T (contract 4096), z = y @ B^T + bias (contract 16+1 via the
ones-row trick), never materializing the dense 4096x4096 W.

Everything on the wire is fp16 (gate is rel_err < 2e-2; measured fp16
end-to-end error ~6e-4): x is cast AND pre-transposed on the host into the
exact SBUF layout [128 partitions = feature%128, (group, k-chunk, row)],
so the device does zero transposes and both GEMMs run at full 16-bit PE
rate with fp32 PSUM accumulation. The output travels back as fp16 and is
upcast on the host. Per-core HBM traffic: 8.4 MiB in + 8.4 MiB out.

Scheduling notes (from trace analysis):
  - Input pieces are enqueued free-running on the SP HWDGE ring; their
    completion semaphores arrive in order at full-bandwidth cadence but
    lag the data by ~3-7 us under load, so the leading pieces are SMALL
    (graduated sizes) to pull the first GEMM1 matmul to ~11 us.
  - Outputs go out on the second HWDGE ring (nc.scalar) so they never
    queue behind inputs (rings are FIFO; SWDGE measured only ~205 GB/s).
  - ~64 tiny garbage matmuls run during the initial DMA wait so the PE
    HAM clock-gate is already released (2.4 GHz) when GEMM1 starts.
  - GEMM1 is split into two column halves; the PSUM->SBUF y^T cast-copy
    of half a overlaps the matmuls of half b, so GEMM2 starts without a
    PE bubble.
  - PSUM->SBUF z copies are [128, 1024] (2 PSUM banks) per instruction,
    alternating DVE/ACT, to amortize per-instruction overhead.
  - bb is loaded as [16, .] + [1, .]: a 17-partition DMA splits unevenly
    over the 16 SDMA engines and its straggler slice completes ~10 us
    late.
"""

import sys

import numpy as np

if "/opt/trn_rl_repo" not in sys.path:
    sys.path.insert(0, "/opt/trn_rl_repo")

import concourse.bass as bass
import concourse.mybir as mybir
from concourse import bacc
from concourse.bass_utils import run_bass_kernel_spmd
from concourse.tile import TileContext

N_CORES = 8
B, S, IN_F, OUT_F, R = 4, 2048, 4096, 4096, 16
ROWS = B * S // N_CORES  # 1024 rows per core
SCALING = 2.0  # alpha / r = 32 / 16, folded into A on the host
FP16 = mybir.dt.float16
FP32 = mybir.dt.float32
P = 128
NK = IN_F // P  # 32 contraction chunks for GEMM1
GROWS = 512  # rows per group (GEMM1 moving free dim)
NG = ROWS // GROWS  # 2 groups per core
HT = GROWS // P  # 4 row-tiles per group
ZC = 512  # GEMM2 moving chunk (PSUM bank = 512 fp32)
NJ = OUT_F // ZC  # 8 output chunks per row tile
YH = GROWS // 2  # GEMM1 column half (256)
# Input pieces in k-chunks, graduated: small leading pieces so GEMM1 can
# start as soon as possible; full-size after. Sums to NG*NK = 64.
PIECE_CHUNKS = [2, 2, 4, 8, 8, 8, 8, 8, 8, 8]
NWARM = 64  # HAM warm-up matmuls

_nc_cache = None


def build_nc() -> bass.Bass:
    assert sum(PIECE_CHUNKS) == NG * NK
    nc = bacc.Bacc()
    x_d = nc.declare_dram_parameter("xt", [P, NG * NK * GROWS], FP16, isOutput=False)
    a_d = nc.declare_dram_parameter("at", [P, NK * R], FP16, isOutput=False)
    bb_d = nc.declare_dram_parameter("bb", [R + 1, OUT_F], FP16, isOutput=False)
    out_d = nc.declare_dram_parameter("out", [ROWS, OUT_F], FP16, isOutput=True)

    with TileContext(nc) as tc:
        with (
            tc.tile_pool(name="const", bufs=1) as const,
            tc.tile_pool(name="xin", bufs=len(PIECE_CHUNKS)) as xin,
            tc.tile_pool(name="zrp", bufs=4) as zrp,
            tc.tile_pool(name="ytp", bufs=2) as ytp,
            tc.tile_pool(name="yapsum", bufs=1, space="PSUM") as yapsum,
            tc.tile_pool(name="ybpsum", bufs=1, space="PSUM") as ybpsum,
            tc.tile_pool(name="zpsum", bufs=3, space="PSUM") as zpsum,
        ):
            # HAM warm-up: keep the PE busy on garbage during the initial
            # DMA wait so real matmuls run at 2.4 GHz, not 1.2.
            warm_sb = const.tile([P, 64], FP16)
            nc.vector.memset(warm_sb[:, :], 0.0)
            warm_ps = zpsum.tile([P, 2 * ZC], FP32, tag="zz")
            for w in range(NWARM):
                nc.tensor.matmul(
                    warm_ps[0:64, 0:64],
                    lhsT=warm_sb[:, 0:64],
                    rhs=warm_sb[:, 0:64],
                    start=(w == 0),
                    stop=(w == NWARM - 1),
                )

            at_sb = const.tile([P, NK * R], FP16)
            nc.sync.dma_start(out=at_sb[:, :], in_=a_d[:, :])
            bb_sb = const.tile([R + 1, OUT_F], FP16)
            nc.sync.dma_start(out=bb_sb[0:R, :], in_=bb_d[0:R, :])
            nc.sync.dma_start(out=bb_sb[R : R + 1, :], in_=bb_d[R : R + 1, :])

            # Free-running input enqueue; chunk_tile[k] -> (tile, col offset)
            chunk_loc = {}
            k0 = 0
            for pi, ck in enumerate(PIECE_CHUNKS):
                pt = xin.tile([P, ck * GROWS], FP16, tag="x", name=f"xp{pi}")
                nc.sync.dma_start(
                    out=pt[:, :],
                    in_=x_d[:, k0 * GROWS : (k0 + ck) * GROWS],
                )
                for kk in range(ck):
                    chunk_loc[k0 + kk] = (pt, kk * GROWS)
                k0 += ck

            for g in range(NG):
                yt = ytp.tile([R + 1, GROWS], FP16, tag="yt")
                nc.vector.memset(yt[:, :], 1.0)
                # GEMM1 in column halves; half-a's copy overlaps half-b's
                # matmuls so GEMM2 starts with no PE bubble.
                for half in range(2):
                    pool = yapsum if half == 0 else ybpsum
                    y_ps = pool.tile([R, YH], FP32, tag=f"y{half}")
                    for k in range(NK):
                        pt, off = chunk_loc[g * NK + k]
                        nc.tensor.matmul(
                            y_ps,
                            lhsT=at_sb[:, k * R : (k + 1) * R],
                            rhs=pt[:, off + half * YH : off + (half + 1) * YH],
                            start=(k == 0),
                            stop=(k == NK - 1),
                        )
                    nc.scalar.copy(
                        out=yt[0:R, half * YH : (half + 1) * YH], in_=y_ps
                    )

                for h in range(HT):
                    zrow = zrp.tile([P, OUT_F], FP16, tag="z")
                    for jp in range(NJ // 2):
                        z_ps = zpsum.tile([P, 2 * ZC], FP32, tag="zz")
                        for sub in range(2):
                            j = 2 * jp + sub
                            nc.tensor.matmul(
                                z_ps[:, sub * ZC : (sub + 1) * ZC],
                                lhsT=yt[:, h * P : (h + 1) * P],
                                rhs=bb_sb[:, j * ZC : (j + 1) * ZC],
                                start=True,
                                stop=True,
                            )
                        dst = zrow[:, jp * 2 * ZC : (jp + 1) * 2 * ZC]
                        if jp % 2 == 0:
                            nc.vector.tensor_copy(out=dst, in_=z_ps[:, :])
                        else:
                            nc.scalar.copy(out=dst, in_=z_ps[:, :])
                    # Second HWDGE ring (ACT) — outputs never queue behind
                    # inputs on the SP ring.
                    nc.scalar.dma_start(
                        out=out_d[(g * HT + h) * P : (g * HT + h + 1) * P, :],
                        in_=zrow[:, :],
                    )

    nc.finalize()
    return nc


def make_in_maps(x, lora_A, lora_B, bias):
    f16 = np.float16
    x2 = np.asarray(x, dtype=np.float32).reshape(B * S, IN_F).astype(f16)
    a2 = (SCALING * np.asarray(lora_A, dtype=np.float32)).astype(f16)
    # at[p, k*16+r] = 2*A[r, k*128+p]
    at = np.ascontiguousarray(
        a2.reshape(R, NK, P).transpose(2, 1, 0).reshape(P, NK * R)
    )
    bb = np.ascontiguousarray(
        np.concatenate(
            [
                np.asarray(lora_B, dtype=np.float32).T.astype(f16),
                np.asarray(bias, dtype=np.float32).astype(f16).reshape(1, OUT_F),
            ],
            axis=0,
        )
    )
    in_maps = []
    for shard in np.split(x2, N_CORES, axis=0):  # [1024, 4096] each
        # xt[p, g, k, r] = shard[g*512 + r, k*128 + p]
        xt = np.ascontiguousarray(
            shard.reshape(NG, GROWS, NK, P)
            .transpose(3, 0, 2, 1)
            .reshape(P, NG * NK * GROWS)
        )
        in_maps.append({"xt": xt, "at": at, "bb": bb})
    return in_maps


def run(inputs: dict, trace: bool = False, **kw):
    global _nc_cache
    if _nc_cache is None:
        _nc_cache = build_nc()
    in_maps = make_in_maps(**inputs)
    res = run_bass_kernel_spmd(
        _nc_cache, in_maps, list(range(N_CORES)), trace=trace, **kw
    )
    out = (
        np.concatenate([res.results[i]["out"] for i in range(N_CORES)], axis=0)
        .astype(np.float32)
        .reshape(B, S, OUT_F)
    )
    return out, res


def kernel(**inputs) -> np.ndarray:
    out, _ = run(inputs)
    return out


# revision 11
# speedup vs baseline: 1.4647x; 1.1319x over previous
"""LoRA linear layer on 8 Trainium2 NeuronCores.

Computes out = x @ (lora_B @ lora_A * 2).T + bias for
x [4, 2048, 4096], lora_A [16, 4096], lora_B [4096, 16], bias [4096].

Strategy: pure data parallel — shard x over batch*seq (8192 rows -> 1024
rows/core), replicate the tiny LoRA weights. Exploit the rank-16 structure:
y = x @ A^T (contract 4096), z = y @ B^T + bias (contract 16+1 via the
ones-row trick), never materializing the dense 4096x4096 W.

Everything on the wire is fp16 (gate is rel_err < 2e-2; measured fp16
end-to-end error ~6e-4): x is cast AND pre-transposed on the host into the
exact SBUF layout [128 partitions = feature%128, (group, k-chunk, row)],
so the device does zero transposes and both GEMMs run at 16-bit PE rate
with fp32 PSUM accumulation. The output travels back as fp16 and is
upcast on the host. Per-core HBM traffic: 8.4 MiB in + 8.4 MiB out.

PE-array tiling (the HAM/power governor holds the PE at 1.2 GHz for most
of the kernel, so streamed cycles are what matters):
  - GEMM1 uses 4x column-tiling: lhsT (A^T chunk) is [128, 16] — only 16
    of 128 array columns. Four concurrent matmuls (tile_position (0,32j))
    each stream a different 128-row subset, so a 512-row group costs
    ~128 cycles per k-chunk instead of 512. Col-tile j's output lands at
    partitions 32j..32j+15 = exactly row-tile j's y^T for GEMM2.
  - GEMM2 uses 2x row-tiling: K=17 occupies only 17 of 128 array rows.
    h-tile pairs run concurrently from base partitions {0,32} / {64,96}
    against a host-replicated BB (= [B^T; bias] at all four 32-groups).

Scheduling notes (from trace analysis):
  - Input pieces are enqueued free-running on the SP HWDGE ring; their
    completion semaphores arrive in order at full-bandwidth cadence but
    lag the data by ~3-7 us under load, so the leading pieces are SMALL
    (graduated sizes) to pull the first GEMM1 matmul to ~11 us.
  - Outputs go out on the second HWDGE ring (nc.scalar) so they never
    queue behind inputs (rings are FIFO; SWDGE measured only ~205 GB/s).
  - ~64 tiny garbage matmuls run during the initial DMA wait to warm the
    PE HAM clock-gate.
  - PSUM->SBUF z copies are [128, 1024] (2 PSUM banks) per instruction,
    alternating DVE/ACT, to amortize per-instruction overhead.
"""

import sys

import numpy as np

if "/opt/trn_rl_repo" not in sys.path:
    sys.path.insert(0, "/opt/trn_rl_repo")

import concourse.bass as bass
import concourse.mybir as mybir
from concourse import bacc
from concourse.bass_utils import run_bass_kernel_spmd
from concourse.tile import TileContext

N_CORES = 8
B, S, IN_F, OUT_F, R = 4, 2048, 4096, 4096, 16
ROWS = B * S // N_CORES  # 1024 rows per core
SCALING = 2.0  # alpha / r = 32 / 16, folded into A on the host
FP16 = mybir.dt.float16
FP32 = mybir.dt.float32
P = 128
NK = IN_F // P  # 32 contraction chunks for GEMM1
GROWS = 512  # rows per group
NG = ROWS // GROWS  # 2 groups per core
HT = GROWS // P  # 4 row-tiles (= col-tiles of GEMM1) per group
ZC = 512  # GEMM2 moving chunk (PSUM bank = 512 fp32)
NJ = OUT_F // ZC  # 8 output chunks per row tile
# Input pieces in k-chunks, graduated: small leading pieces so GEMM1 can
# start as soon as possible; full-size after. Sums to NG*NK = 64.
PIECE_CHUNKS = [2, 2, 4, 8, 8, 8, 8, 8, 8, 8]
NWARM = 64  # HAM warm-up matmuls

_nc_cache = None


def build_nc() -> bass.Bass:
    assert sum(PIECE_CHUNKS) == NG * NK
    nc = bacc.Bacc()
    x_d = nc.declare_dram_parameter("xt", [P, NG * NK * GROWS], FP16, isOutput=False)
    a_d = nc.declare_dram_parameter("at", [P, NK * R], FP16, isOutput=False)
    bb_d = nc.declare_dram_parameter("bb4", [P, OUT_F], FP16, isOutput=False)
    out_d = nc.declare_dram_parameter("out", [ROWS, OUT_F], FP16, isOutput=True)

    with TileContext(nc) as tc:
        with (
            tc.tile_pool(name="const", bufs=1) as const,
            tc.tile_pool(name="xin", bufs=len(PIECE_CHUNKS)) as xin,
            tc.tile_pool(name="zrp", bufs=4) as zrp,
            tc.tile_pool(name="ytp", bufs=2) as ytp,
            tc.tile_pool(name="ypsum", bufs=2, space="PSUM") as ypsum,
            tc.tile_pool(name="zpsum", bufs=3, space="PSUM") as zpsum,
        ):
            # HAM warm-up: keep the PE busy on garbage during the initial
            # DMA wait so real matmuls run at 2.4 GHz, not 1.2.
            warm_sb = const.tile([P, 64], FP16)
            nc.vector.memset(warm_sb[:, :], 0.0)
            warm_ps = zpsum.tile([P, 2 * ZC], FP32, tag="zz")
            for w in range(NWARM):
                nc.tensor.matmul(
                    warm_ps[0:64, 0:64],
                    lhsT=warm_sb[:, 0:64],
                    rhs=warm_sb[:, 0:64],
                    start=(w == 0),
                    stop=(w == NWARM - 1),
                )

            at_sb = const.tile([P, NK * R], FP16)
            nc.sync.dma_start(out=at_sb[:, :], in_=a_d[:, :])
            bb_sb = const.tile([P, OUT_F], FP16)
            nc.sync.dma_start(out=bb_sb[:, :], in_=bb_d[:, :])

            # Free-running input enqueue; chunk -> (tile, col offset)
            chunk_loc = {}
            k0 = 0
            for pi, ck in enumerate(PIECE_CHUNKS):
                pt = xin.tile([P, ck * GROWS], FP16, tag="x", name=f"xp{pi}")
                nc.sync.dma_start(
                    out=pt[:, :],
                    in_=x_d[:, k0 * GROWS : (k0 + ck) * GROWS],
                )
                for kk in range(ck):
                    chunk_loc[k0 + kk] = (pt, kk * GROWS)
                k0 += ck

            for g in range(NG):
                # GEMM1, 4x col-tiled: y_ps[32j+r, n] = y^T[r, 128j+n].
                y_ps = ypsum.tile([P, P], FP32, tag="y")
                for k in range(NK):
                    pt, off = chunk_loc[g * NK + k]
                    for j in range(HT):
                        nc.tensor.matmul(
                            y_ps[32 * j : 32 * j + R, :],
                            lhsT=at_sb[:, k * R : (k + 1) * R],
                            rhs=pt[:, off + j * P : off + (j + 1) * P],
                            start=(k == 0),
                            stop=(k == NK - 1),
                            tile_position=(0, 32 * j),
                            skip_group_check=True,
                        )

                # yt[32h+r, :] = y^T rows for h-tile h; row 32h+16 = ones.
                yt = ytp.tile([P, P], FP16, tag="yt")
                nc.vector.memset(yt[:, :], 1.0)
                for h in range(HT):
                    src = y_ps[32 * h : 32 * h + R, :]
                    dst = yt[32 * h : 32 * h + R, :]
                    if h % 2 == 0:
                        nc.scalar.copy(out=dst, in_=src)
                    else:
                        nc.vector.tensor_copy(out=dst, in_=src)

                # GEMM2, 2x row-tiled: h-pairs (0,1) and (2,3) concurrent.
                for hp in range(HT // 2):
                    h0, h1 = 2 * hp, 2 * hp + 1
                    zrow0 = zrp.tile([P, OUT_F], FP16, tag="z", name="zr0")
                    zrow1 = zrp.tile([P, OUT_F], FP16, tag="z", name="zr1")
                    for jp in range(NJ // 2):
                        za = zpsum.tile([P, 2 * ZC], FP32, tag="zz", name="za")
                        zb = zpsum.tile([P, 2 * ZC], FP32, tag="zz", name="zb")
                        for sub in range(2):
                            j = 2 * jp + sub
                            for h, zt in ((h0, za), (h1, zb)):
                                nc.tensor.matmul(
                                    zt[:, sub * ZC : (sub + 1) * ZC],
                                    lhsT=yt[32 * h : 32 * h + R + 1, :],
                                    rhs=bb_sb[
                                        32 * h : 32 * h + R + 1,
                                        j * ZC : (j + 1) * ZC,
                                    ],
                                    start=True,
                                    stop=True,
                                    tile_position=(32 * h, 0),
                                )
                        dsl = slice(jp * 2 * ZC, (jp + 1) * 2 * ZC)
                        nc.vector.tensor_copy(out=zrow0[:, dsl], in_=za[:, :])
                        nc.scalar.copy(out=zrow1[:, dsl], in_=zb[:, :])
                    for h, zrow in ((h0, zrow0), (h1, zrow1)):
                        nc.scalar.dma_start(
                            out=out_d[(g * HT + h) * P : (g * HT + h + 1) * P, :],
                            in_=zrow[:, :],
                        )

    nc.finalize()
    return nc


def make_in_maps(x, lora_A, lora_B, bias):
    f16 = np.float16
    x2 = np.asarray(x, dtype=np.float32).reshape(B * S, IN_F).astype(f16)
    a2 = (SCALING * np.asarray(lora_A, dtype=np.float32)).astype(f16)
    # at[p, k*16+r] = 2*A[r, k*128+p]
    at = np.ascontiguousarray(
        a2.reshape(R, NK, P).transpose(2, 1, 0).reshape(P, NK * R)
    )
    # bb4: [B^T; bias] replicated at partition groups 0/32/64/96.
    bb4 = np.zeros((P, OUT_F), dtype=f16)
    bt = np.asarray(lora_B, dtype=np.float32).T.astype(f16)  # [R, OUT_F]
    bs = np.asarray(bias, dtype=np.float32).astype(f16)
    for gpart in range(4):
        bb4[32 * gpart : 32 * gpart + R, :] = bt
        bb4[32 * gpart + R, :] = bs
    in_maps = []
    for shard in np.split(x2, N_CORES, axis=0):  # [1024, 4096] each
        # xt[p, g, k, r] = shard[g*512 + r, k*128 + p]
        xt = np.ascontiguousarray(
            shard.reshape(NG, GROWS, NK, P)
            .transpose(3, 0, 2, 1)
            .reshape(P, NG * NK * GROWS)
        )
        in_maps.append({"xt": xt, "at": at, "bb4": bb4})
    return in_maps


def run(inputs: dict, trace: bool = False, **kw):
    global _nc_cache
    if _nc_cache is None:
        _nc_cache = build_nc()
    in_maps = make_in_maps(**inputs)
    res = run_bass_kernel_spmd(
        _nc_cache, in_maps, list(range(N_CORES)), trace=trace, **kw
    )
    out = (
        np.concatenate([res.results[i]["out"] for i in range(N_CORES)], axis=0)
        .astype(np.float32)
        .reshape(B, S, OUT_F)
    )
    return out, res


def kernel(**inputs) -> np.ndarray:
    out, _ = run(inputs)
    return out
